# revision 53
# baseline (speedup 1.0000x reference)
"""Trainium2 Bass kernel for 3-layer HypergraphConv (PyG HypergraphConv, no attention).

Math per layer (indices fixed across layers):
    e   = (Binv * segsum_edge(h[node_idx])) @ W          # [M, C]
    h'  = relu(Dinv * segsum_node(e[edge_idx]) + b)      # [N, C]
with D = segsum_node(hw[edge_idx]), Bc = segsum_edge(1), safe-inverted.

Device strategy (8 NeuronCores, SPMD single NEFF):
  - pass1: shard destination EDGES (M/8 per core); entries sorted by dest edge
    block; gather source rows with dma_gather (custom InstDMAGatherAnt,
    int16 indices), segment-sum via one-hot matmuls accumulated in PSUM
    (one-hot built on DVE by comparing a host-provided iota tile with
    per-entry local-dest ids).
  - pass2: shard destination NODES (N/8 per core), same machinery, source = e.
  - Collectives are CHUNKED and interleaved with the gather pipeline: the
    per-core shard of e (resp. h) is split into 4 stripe chunks; each chunk's
    AllGather is issued as soon as its destination blocks finish, and the next
    pass's gather jobs are grouped by source stripe so they only wait on their
    own chunk. This hides nearly all collective latency behind the
    descriptor-generation-bound dma_gather stream (the kernel's hard floor:
    ~8.5ns per gathered row on the SWDGE path).
  - Final layer writes the node shard; host concats.

Host-side work is limited to index manipulation (sorting/packing/relayout)
and the degree vectors (bincount) -- all feature math runs on device.
"""

import sys
import numpy as np
from contextlib import ExitStack

for _p in ("/opt/trn_rl_repo",):
    if _p not in sys.path:
        sys.path.insert(0, _p)

import concourse.bass as bass
import concourse.bacc as bacc
import concourse.mybir as mybir
from concourse import bass_utils
from concourse.tile import TileContext
from concourse.mybir import AluOpType

F32 = mybir.dt.float32
BF16 = mybir.dt.bfloat16
I16 = mybir.dt.int16

# full-size problem constants (hardcoded; kernel.py must be self-contained)
N_FULL, M_FULL, C_FULL, NDEV = 100000, 20000, 128, 8
SRC_GROUP_FULL = 25000      # kept for test.py signature compat (unused)
GCAP = 16                   # max 128-entry slots per dma_gather (2048 entries)
PAD_LSEG = 1.0e9            # local-seg value for padding entries (matches no iota col)

# stripe chunking: per-core local row ranges, block-aligned (128-row blocks).
# Edge chunks are uneven: the big chunk's AllGather hides behind the tail of
# pass1; the tiny last chunk minimizes the exposed latency pass2 waits on.
NODE_CHUNK_BLOCKS = [30, 30, 30, 8]           # 98 blocks over 12500 local nodes
EDGE_CHUNK_BLOCKS = [16, 4]                   # 20 blocks over 2500 local edges
NCC = len(NODE_CHUNK_BLOCKS)
ECC = len(EDGE_CHUNK_BLOCKS)
# block index -> chunk, and last block of each chunk, per side
EBLK_OF, EBLK_LAST = [], []
for _c, _nb in enumerate(EDGE_CHUNK_BLOCKS):
    EBLK_OF += [_c] * _nb
    EBLK_LAST.append(len(EBLK_OF) - 1)
NBLK_OF, NBLK_LAST = [], []
for _c, _nb in enumerate(NODE_CHUNK_BLOCKS):
    NBLK_OF += [_c] * _nb
    NBLK_LAST.append(len(NBLK_OF) - 1)


def _chunk_rows(chunk_blocks, n_local):
    rows, acc = [], 0
    for nb in chunk_blocks:
        r = min(nb * 128, n_local - acc)
        rows.append(r)
        acc += r
    assert acc == n_local
    return rows


def _make_src_maps(n_local, ndev, chunk_rows_):
    """Global source id -> (group, row within group tensor).

    Group tensor g is the AllGather of every device's local stripe g:
    rows ordered [dev0 stripe, dev1 stripe, ...]."""
    starts = np.concatenate(([0], np.cumsum(chunk_rows_)[:-1]))
    cmap = np.zeros(n_local, np.int64)
    rmap = np.zeros(n_local, np.int64)
    for c, (s, r) in enumerate(zip(starts, chunk_rows_)):
        cmap[s:s + r] = c
        rmap[s:s + r] = np.arange(r)
    crows = np.asarray(chunk_rows_, np.int64)

    def src_grp(s):
        return cmap[s % n_local]

    def src_row(s):
        l = s % n_local
        return (s // n_local) * crows[cmap[l]] + rmap[l]

    grp_rows = [int(r) * ndev for r in chunk_rows_]
    return src_grp, src_row, grp_rows


def _make_src_maps_fused(n_local, ndev, chunk_rows_):
    """Like _make_src_maps but all chunks live in ONE tensor, concatenated:
    [chunk0: dev-major rows, chunk1: dev-major rows, ...]. Single group."""
    starts = np.concatenate(([0], np.cumsum(chunk_rows_)[:-1]))
    cmap = np.zeros(n_local, np.int64)
    rmap = np.zeros(n_local, np.int64)
    for c, (s, r) in enumerate(zip(starts, chunk_rows_)):
        cmap[s:s + r] = c
        rmap[s:s + r] = np.arange(r)
    crows = np.asarray(chunk_rows_, np.int64)
    coffs = np.concatenate(([0], np.cumsum(crows * ndev)[:-1]))

    def src_grp(s):
        return np.zeros(np.shape(s), np.int64)

    def src_row(s):
        l = s % n_local
        c = cmap[l]
        return coffs[c] + (s // n_local) * crows[c] + rmap[l]

    return src_grp, src_row, [n_local * ndev]


class PassPlan:
    """Uniform-across-devices plan for one segment-sum pass.

    dest_idx sharded into contiguous per-device ranges; sources assigned to
    groups via src_grp/src_row maps (group tensors hold AllGathered stripes)."""

    def __init__(self, dest_idx, src_idx, n_dest, ndev, src_grp, src_row,
                 grp_rows):
        dpd = n_dest // ndev
        nblk = -(-dpd // 128)
        ngrp = len(grp_rows)
        self.dpd, self.nblk, self.ngrp = dpd, nblk, ngrp
        self.grp_rows = grp_rows
        counts = np.zeros((ndev, nblk, ngrp), np.int64)
        orders = []
        for d in range(ndev):
            m0 = d * dpd
            sel = np.nonzero((dest_idx >= m0) & (dest_idx < m0 + dpd))[0]
            de = (dest_idx[sel] - m0).astype(np.int64)
            sr = src_idx[sel].astype(np.int64)
            blk = de >> 7
            grp = src_grp(sr)
            row = src_row(sr)
            key = blk * ngrp + grp
            o = np.argsort(key, kind="stable")
            de, row, blk, grp, key = de[o], row[o], blk[o], grp[o], key[o]
            counts[d] = np.bincount(key, minlength=nblk * ngrp).reshape(nblk, ngrp)
            orders.append((de, row, blk, grp, key))
        uslots = -(-counts.max(axis=0) // 128)          # [nblk, ngrp]
        jobs = []                                       # (block, group, slot_off, n_slots)
        blocks = []                                     # (job_lo, job_hi, n_valid_dest)
        soff = 0
        base = np.zeros((nblk, ngrp), np.int64)
        for b in range(nblk):
            j0 = len(jobs)
            for g in range(ngrp):
                base[b, g] = soff
                rem = int(uslots[b, g])
                while rem > 0:
                    take = min(rem, GCAP)
                    jobs.append((b, g, soff, take))
                    soff += take
                    rem -= take
            blocks.append((j0, len(jobs), min(128, dpd - b * 128)))
        self.jobs, self.blocks = jobs, blocks
        self.total_slots = soff
        L = soff * 128
        self.L = L
        self.idx_w = np.zeros((ndev, 128, L // 16), np.int16)
        self.lseg_w = np.full((ndev, 128, L // 128), PAD_LSEG, np.float32)
        for d in range(ndev):
            de, row, blk, grp, key = orders[d]
            idx_flat = np.zeros(L, np.int16)
            lseg_flat = np.full(L, PAD_LSEG, np.float32)
            first = np.zeros(nblk * ngrp + 1, np.int64)
            np.cumsum(counts[d].ravel(), out=first[1:])
            rank = np.arange(len(de)) - first[key]
            pos = base[blk, grp] * 128 + rank
            assert row.size == 0 or row.max() < 32768
            idx_flat[pos] = row.astype(np.int16)
            lseg_flat[pos] = (de & 127).astype(np.float32)
            self.idx_w[d] = np.tile(np.ascontiguousarray(idx_flat.reshape(-1, 16).T), (8, 1))
            self.lseg_w[d] = np.ascontiguousarray(lseg_flat.reshape(-1, 128).T)


class ChainPlan:
    """Chain-packed plan for a single-source-group pass: entries packed
    consecutively with NO per-block slot padding. A slot whose 128 lanes
    straddle a block boundary (per-device boundaries differ) is matmul'd once
    per covered block ("variant"), with a separate one-hot column per variant
    (entries outside the variant's block get lseg=PAD -> zero one-hot)."""

    def __init__(self, dest_idx, src_idx, n_dest, ndev, src_row, grp_rows):
        dpd = n_dest // ndev
        nblk = -(-dpd // 128)
        self.dpd, self.nblk, self.ngrp = dpd, nblk, 1
        self.grp_rows = grp_rows
        per_dev = []
        cnts = []
        for d in range(ndev):
            m0 = d * dpd
            sel = np.nonzero((dest_idx >= m0) & (dest_idx < m0 + dpd))[0]
            de = (dest_idx[sel] - m0).astype(np.int64)
            row = src_row(src_idx[sel].astype(np.int64))
            o = np.argsort(de >> 7, kind="stable")
            per_dev.append((de[o], row[o]))
            cnts.append(len(de))
        S = -(-max(cnts) // 128)                 # slots in the chain
        # per-slot block span (union over devices)
        b_lo = np.full(S, nblk, np.int64)
        b_hi = np.full(S, -1, np.int64)
        for d in range(ndev):
            de, _ = per_dev[d]
            blk = de >> 7
            for s in range(-(-cnts[d] // 128)):
                seg = blk[s * 128:(s + 1) * 128]
                b_lo[s] = min(b_lo[s], seg[0])
                b_hi[s] = max(b_hi[s], seg[-1])
        b_lo = np.minimum(b_lo, np.where(b_hi >= 0, b_hi, nblk))
        # variant columns (slot, block), slot-major; jobs = GCAP-slot gathers
        cols = []
        for s in range(S):
            if b_hi[s] < 0:
                cols.append((s, -1))             # all-pad slot: one dead column
            else:
                for b in range(int(b_lo[s]), int(b_hi[s]) + 1):
                    cols.append((s, b))
        self.cols = cols
        # per-block first/last column index (for psum start/stop + epilogue)
        self.blk_cols = [[] for _ in range(nblk)]
        for k, (s, b) in enumerate(cols):
            if b >= 0:
                self.blk_cols[b].append(k)
        for b in range(nblk):
            assert self.blk_cols[b], f"block {b} got no columns"
        self.blocks = [(min(128, dpd - b * 128)) for b in range(nblk)]
        self.jobs = [(s0, min(GCAP, S - s0)) for s0 in range(0, S, GCAP)]
        self.total_slots = S
        L = S * 128
        self.L = L
        ncol = len(cols)
        self.ncol = ncol
        self.idx_w = np.zeros((ndev, 128, L // 16), np.int16)
        self.lseg_w = np.full((ndev, 128, ncol), PAD_LSEG, np.float32)
        for d in range(ndev):
            de, row = per_dev[d]
            n = len(de)
            idx_flat = np.zeros(L, np.int16)
            assert row.size == 0 or row.max() < 32768
            idx_flat[:n] = row.astype(np.int16)
            self.idx_w[d] = np.tile(
                np.ascontiguousarray(idx_flat.reshape(-1, 16).T), (8, 1))
            blk_full = np.full(L, -9, np.int64)
            de_full = np.zeros(L, np.int64)
            blk_full[:n] = de >> 7
            de_full[:n] = de & 127
            lg = self.lseg_w[d]
            for k, (s, b) in enumerate(cols):
                if b < 0:
                    continue
                seg_blk = blk_full[s * 128:(s + 1) * 128]
                seg_de = de_full[s * 128:(s + 1) * 128]
                lg[:, k] = np.where(seg_blk == b, seg_de, PAD_LSEG)


def _pack_per_part(vec, dpd, ndev, nblk):
    """[n] -> [ndev, 128, nblk] with vec[d*dpd + b*128 + p]."""
    out = np.zeros((ndev, 128, nblk), np.float32)
    for d in range(ndev):
        v = vec[d * dpd:(d + 1) * dpd]
        v = np.pad(v, (0, nblk * 128 - dpd))
        out[d] = v.reshape(nblk, 128).T
    return out


def _bcast_free(ap_2d, n):
    """Append a broadcast (step 0) innermost free dim of size n to a 2D AP."""
    return bass.AP(ap_2d.tensor, ap_2d.offset, list(ap_2d.ap) + [[0, n]])


def _view3(ap_2d, inner):
    """View [P, a*inner] 2D AP as [P, a, inner]."""
    p, f = ap_2d.ap[0], ap_2d.ap[1]
    assert f[0] == 1 and f[1] % inner == 0
    return bass.AP(ap_2d.tensor, ap_2d.offset, [p, [inner, f[1] // inner], [1, inner]])


def build_program(nc, p1, p2, n_nodes, n_edges, C, ndev, nlayers, debug=""):
    dpd1, dpd2 = p1.dpd, p2.dpd
    erows = _chunk_rows(EDGE_CHUNK_BLOCKS, dpd1)
    nrows = _chunk_rows(NODE_CHUNK_BLOCKS, dpd2)
    eblk0 = np.concatenate(([0], np.cumsum(EDGE_CHUNK_BLOCKS)[:-1]))
    nblk0 = np.concatenate(([0], np.cumsum(NODE_CHUNK_BLOCKS)[:-1]))
    din = {}

    def ein(name, shape, dtype=F32):
        din[name] = nc.dram_tensor(name, list(shape), dtype, kind="ExternalInput")
        return din[name]

    x_d = [ein(f"xc{c}", [p1.grp_rows[c], C]) for c in range(NCC)]
    idx1_d = ein("idx1", [128, p1.L // 16], I16)
    lseg1_d = ein("lseg1", [128, p1.L // 128])
    idx2_d = ein("idx2", [128, p2.L // 16], I16)
    lseg2_d = ein("lseg2", [128, p2.ncol])
    binv_d = ein("binv", [128, p1.nblk])
    dinv_d = ein("dinv", [128, p2.nblk])
    iota_d = ein("iota", [128, 512])
    ident_d = ein("ident", [128, 128])
    w_d = [ein(f"w{l}", [C, C]) for l in range(nlayers)]
    bt_d = [ein(f"bt{l}", [128, C]) for l in range(nlayers)]
    out_d = nc.dram_tensor("out", [dpd2, C], F32, kind="ExternalOutput")

    with ExitStack() as ctx:
        tc = ctx.enter_context(TileContext(nc))
        cpool = ctx.enter_context(tc.tile_pool(name="consts", bufs=1))
        dpool = ctx.enter_context(tc.tile_pool(name="dram", bufs=1, space="DRAM"))
        gpool = ctx.enter_context(tc.tile_pool(name="gather", bufs=6))
        opool = ctx.enter_context(tc.tile_pool(name="onehot", bufs=8))
        pspool = ctx.enter_context(tc.tile_pool(name="mmps", bufs=4, space="PSUM"))
        eppool = ctx.enter_context(tc.tile_pool(name="ep", bufs=4))
        ep_ps = ctx.enter_context(tc.tile_pool(name="epps", bufs=2, space="PSUM"))

        def load_const(dram, tag):
            t = cpool.tile(dram.shape, dram.dtype, tag=tag, name=tag)
            nc.sync.dma_start(t[:, :], dram.ap())
            return t

        idx1_sb = load_const(idx1_d, "idx1")
        lseg1_sb = load_const(lseg1_d, "lseg1")
        idx2_sb = load_const(idx2_d, "idx2")
        lseg2_sb = load_const(lseg2_d, "lseg2")
        binv_sb = load_const(binv_d, "binv")
        dinv_sb = load_const(dinv_d, "dinv")
        iota_sb = load_const(iota_d, "iota")
        ident_sb = load_const(ident_d, "ident")
        w_sb = [load_const(w_d[l], f"w{l}") for l in range(nlayers)]
        bt_sb = [load_const(bt_d[l], f"bt{l}") for l in range(nlayers)]

        h_src = [x_d[c].ap() for c in range(NCC)]

        def seg_sum_pass(plan, idx_sb, lseg_sb, src_aps, epilogue, after_block,
                         dt=F32):
            dtag = "f" if dt == F32 else "b"
            for b, (j0, j1, vb) in enumerate(plan.blocks):
                ps = pspool.tile([128, 128], F32, tag="ps", name="ps")
                nchunks = sum(plan.jobs[j][3] for j in range(j0, j1))
                ci = 0
                for j in range(j0, j1):
                    _, g, soff, ns = plan.jobs[j]
                    gt = gpool.tile([128, GCAP, 128], dt, tag=f"gt{dtag}",
                                    name="gt")
                    nc.gpsimd.dma_gather(
                        gt[:, :ns, :],
                        src_aps[g],
                        idx_sb[:, soff * 8:(soff + ns) * 8],
                        ns * 128,
                        ns * 128,
                        128,
                        single_packet=(ns * 128 <= 1024),
                    )
                    for q0 in range(0, ns, 4):
                        qs = min(4, ns - q0)
                        oh = opool.tile([128, 512], dt, tag=f"oh{dtag}",
                                        name="oh")
                        oh3 = _view3(oh[:, :qs * 128], 128)
                        iota3 = _view3(iota_sb[:, :qs * 128], 128)
                        ls3 = _bcast_free(lseg_sb[:, soff + q0:soff + q0 + qs], 128)
                        nc.vector.tensor_tensor(oh3, iota3, ls3, AluOpType.is_equal)
                        for k in range(qs):
                            nc.tensor.matmul(
                                ps[:, :],
                                lhsT=oh[:, k * 128:(k + 1) * 128],
                                rhs=gt[:, q0 + k, :],
                                start=(ci == 0),
                                stop=(ci == nchunks - 1),
                            )
                            ci += 1
                epilogue(b, vb, ps)
                after_block(b)

        def chain_pass(plan, idx_sb, lseg_sb, src_ap, epilogue, after_block,
                       dt=F32):
            dtag = "f" if dt == F32 else "b"
            cols = plan.cols
            first_col = {b: plan.blk_cols[b][0] for b in range(plan.nblk)}
            last_col = {b: plan.blk_cols[b][-1] for b in range(plan.nblk)}
            ps_of = {}
            k = 0
            for (s0, ns) in plan.jobs:
                gt = gpool.tile([128, GCAP, 128], dt, tag=f"gt{dtag}",
                                name="gt")
                nc.gpsimd.dma_gather(
                    gt[:, :ns, :],
                    src_ap,
                    idx_sb[:, s0 * 8:(s0 + ns) * 8],
                    ns * 128,
                    ns * 128,
                    128,
                    single_packet=(ns * 128 <= 1024),
                )
                k1 = k
                while k1 < len(cols) and cols[k1][0] < s0 + ns:
                    k1 += 1
                while k < k1:
                    qs = min(4, k1 - k)
                    oh = opool.tile([128, 512], dt, tag=f"oh{dtag}", name="oh")
                    oh3 = _view3(oh[:, :qs * 128], 128)
                    iota3 = _view3(iota_sb[:, :qs * 128], 128)
                    ls3 = _bcast_free(lseg_sb[:, k:k + qs], 128)
                    nc.vector.tensor_tensor(oh3, iota3, ls3, AluOpType.is_equal)
                    for q in range(qs):
                        s, b = cols[k]
                        if b >= 0:
                            if k == first_col[b]:
                                ps_of[b] = pspool.tile([128, 128], F32,
                                                       tag="ps", name="ps")
                            nc.tensor.matmul(
                                ps_of[b][:, :],
                                lhsT=oh[:, q * 128:(q + 1) * 128],
                                rhs=gt[:, s - s0, :],
                                start=(k == first_col[b]),
                                stop=(k == last_col[b]),
                                skip_group_check=True,
                            )
                            if k == last_col[b]:
                                epilogue(b, plan.blocks[b], ps_of.pop(b))
                                after_block(b)
                        k += 1

        scope = nc.named_scope
        for layer in range(nlayers):
            shared_kw = {} if debug == "nocc" else {"addr_space": "Shared"}
            e_shard = [dpool.tile([erows[c], C], BF16, tag=f"es{layer}_{c}",
                                  name=f"es{layer}_{c}") for c in range(ECC)]
            # plain DRAM (not Shared): two chunked AllGathers write disjoint
            # row slices, and Shared scratch enforces a single writer
            e_full = dpool.tile([dpd1 * ndev, C], BF16, tag=f"ef{layer}",
                                name=f"ef{layer}")
            ecoff = np.concatenate(([0], np.cumsum(
                [erows[c] * ndev for c in range(ECC)])[:-1]))

            def ep1(b, vb, ps, _l=layer, _es=e_shard):
                c = EBLK_OF[b]
                r0 = (b - int(eblk0[c])) * 128
                tmp = eppool.tile([128, 128], F32, tag="tmp", name="tmp")
                nc.vector.tensor_scalar(tmp[:, :], ps[:, :], binv_sb[:, b:b + 1], None,
                                        AluOpType.mult)
                psT = ep_ps.tile([128, 128], F32, tag="psT", name="psT")
                nc.tensor.transpose(psT[:, :], tmp[:, :], ident_sb[:, :])
                tmpT = eppool.tile([128, 128], F32, tag="tmpT", name="tmpT")
                nc.scalar.copy(tmpT[:, :], psT[:, :])
                psE = ep_ps.tile([128, 128], F32, tag="psE", name="psE")
                nc.tensor.matmul(psE[:, :], lhsT=tmpT[:, :], rhs=w_sb[_l][:, :],
                                 start=True, stop=True)
                et = eppool.tile([128, 128], BF16, tag="et", name="et")
                nc.scalar.copy(et[:, :], psE[:, :])
                nc.sync.dma_start(_es[c][r0:r0 + vb, :], et[:vb, :])

            def cc_e(b, _l=layer, _es=e_shard, _ef=e_full, _ecoff=ecoff):
                if b not in EBLK_LAST:
                    return
                c = EBLK_LAST.index(b)
                o = int(_ecoff[c])
                with scope(f"L{_l}_ccE{c}"):
                    if debug == "nocc":
                        for bb in range(0, erows[c], 128):
                            vbb = min(128, erows[c] - bb)
                            tcp = eppool.tile([128, 128], BF16, tag="tcp", name="tcp")
                            nc.sync.dma_start(tcp[:vbb, :], _es[c][bb:bb + vbb, :])
                            nc.sync.dma_start(_ef[o + bb:o + bb + vbb, :], tcp[:vbb, :])
                    else:
                        nc.gpsimd.collective_compute(
                            "AllGather", AluOpType.bypass,
                            replica_groups=[list(range(ndev))],
                            ins=[_es[c][:, :]],
                            outs=[_ef[o:o + erows[c] * ndev, :]],
                        )

            with scope(f"L{layer}_p1"):
                seg_sum_pass(p1, idx1_sb, lseg1_sb, h_src, ep1, cc_e,
                             dt=(F32 if layer == 0 else BF16))

            last = layer == nlayers - 1
            if not last:
                h_shard = [dpool.tile([nrows[c], C], BF16, tag=f"hs{layer}_{c}",
                                      name=f"hs{layer}_{c}") for c in range(NCC)]
                h_full = [dpool.tile([nrows[c] * ndev, C], BF16, tag=f"hf{layer}_{c}",
                                     name=f"hf{layer}_{c}", **shared_kw)
                          for c in range(NCC)]

            def ep2(b, vb, ps, _l=layer, _last=last,
                    _hs=None if last else h_shard):
                tmp = eppool.tile([128, 128], F32, tag="tmp2", name="tmp2")
                nc.vector.tensor_scalar(tmp[:, :], ps[:, :], dinv_sb[:, b:b + 1], None,
                                        AluOpType.mult)
                tmp2 = eppool.tile([128, 128], F32, tag="tmp3", name="tmp3")
                nc.vector.tensor_tensor(tmp2[:, :], tmp[:, :], bt_sb[_l][:, :],
                                        AluOpType.add)
                ht = eppool.tile([128, 128], F32 if _last else BF16,
                                 tag="htf" if _last else "htb", name="ht")
                nc.vector.tensor_scalar(ht[:, :], tmp2[:, :], 0.0, None, AluOpType.max)
                if _last:
                    nc.sync.dma_start(out_d.ap()[b * 128:b * 128 + vb, :], ht[:vb, :])
                else:
                    c = NBLK_OF[b]
                    r0 = (b - int(nblk0[c])) * 128
                    nc.sync.dma_start(_hs[c][r0:r0 + vb, :], ht[:vb, :])

            def cc_h(b, _l=layer, _last=last,
                     _hs=None if last else h_shard,
                     _hf=None if last else h_full):
                if _last:
                    return
                if b not in NBLK_LAST:
                    return
                c = NBLK_LAST.index(b)
                with scope(f"L{_l}_ccH{c}"):
                    if debug == "nocc":
                        for bb in range(0, nrows[c], 128):
                            vbb = min(128, nrows[c] - bb)
                            tcp = eppool.tile([128, 128], BF16, tag="tcp", name="tcp")
                            nc.sync.dma_start(tcp[:vbb, :], _hs[c][bb:bb + vbb, :])
                            nc.sync.dma_start(_hf[c][bb:bb + vbb, :], tcp[:vbb, :])
                    else:
                        nc.gpsimd.collective_compute(
                            "AllGather", AluOpType.bypass,
                            replica_groups=[list(range(ndev))],
                            ins=[_hs[c][:, :]], outs=[_hf[c][:, :]],
                        )

            with scope(f"L{layer}_p2"):
                chain_pass(p2, idx2_sb, lseg2_sb, e_full[:, :], ep2, cc_h,
                           dt=BF16)

            if not last:
                h_src = [hf[:, :] for hf in h_full]

    return din, out_d


def make_plans(node_idx, edge_idx, n_nodes, n_edges, ndev=NDEV):
    dpd1 = n_edges // ndev
    dpd2 = n_nodes // ndev
    erows = _chunk_rows(EDGE_CHUNK_BLOCKS, dpd1)
    nrows = _chunk_rows(NODE_CHUNK_BLOCKS, dpd2)
    ngrp_src1, nrow_src1, grp_rows1 = _make_src_maps(dpd2, ndev, nrows)
    egrp_src2, erow_src2, grp_rows2 = _make_src_maps_fused(dpd1, ndev, erows)
    p1 = PassPlan(edge_idx, node_idx, n_edges, ndev, ngrp_src1, nrow_src1,
                  grp_rows1)
    p2 = ChainPlan(node_idx, edge_idx, n_nodes, ndev, erow_src2, grp_rows2)
    return p1, p2, nrows


def stage_x(x, nrows, ndev=NDEV):
    """Relayout x into the 4 stripe-group tensors matching AllGather layout."""
    dpd2 = x.shape[0] // ndev
    xr = np.ascontiguousarray(x).reshape(ndev, dpd2, x.shape[1])
    starts = np.concatenate(([0], np.cumsum(nrows)[:-1]))
    return [np.ascontiguousarray(
        xr[:, int(s):int(s) + int(r), :].reshape(ndev * int(r), x.shape[1]))
        for s, r in zip(starts, nrows)]


def run(x, node_idx, edge_idx, hw, Ws, bs, n_nodes, n_edges, src_group=None,
        ndev=NDEV, trace=False, debug=""):
    C = x.shape[1]
    nlayers = len(Ws)
    assert n_nodes % ndev == 0 and n_edges % ndev == 0

    p1, p2, nrows = make_plans(node_idx, edge_idx, n_nodes, n_edges, ndev)

    Bc = np.bincount(edge_idx, minlength=n_edges).astype(np.float32)
    D = np.bincount(node_idx, weights=hw[edge_idx].astype(np.float64),
                    minlength=n_nodes).astype(np.float32)
    Binv = np.where(Bc > 0, 1.0 / np.where(Bc > 0, Bc, 1), 0).astype(np.float32)
    Dinv = np.where(D > 0, 1.0 / np.where(D > 0, D, 1), 0).astype(np.float32)
    binv_pp = _pack_per_part(Binv, p1.dpd, ndev, p1.nblk)
    dinv_pp = _pack_per_part(Dinv, p2.dpd, ndev, p2.nblk)

    iota = np.tile(np.arange(128, dtype=np.float32), (128, 4)).reshape(128, 512)
    ident = np.eye(128, dtype=np.float32)
    xcs = stage_x(np.asarray(x, np.float32), nrows, ndev)

    nc = bacc.Bacc("TRN2", target_bir_lowering=False, debug=False,
                   num_devices=ndev)
    build_program(nc, p1, p2, n_nodes, n_edges, C, ndev, nlayers, debug=debug)
    nc.compile()

    in_maps = []
    for d in range(ndev):
        m = {
            "idx1": p1.idx_w[d], "lseg1": p1.lseg_w[d],
            "idx2": p2.idx_w[d], "lseg2": p2.lseg_w[d],
            "binv": binv_pp[d], "dinv": dinv_pp[d],
            "iota": iota, "ident": ident,
        }
        for c in range(len(xcs)):
            m[f"xc{c}"] = xcs[c]
        for l in range(nlayers):
            m[f"w{l}"] = np.ascontiguousarray(Ws[l].astype(np.float32))
            m[f"bt{l}"] = np.tile(bs[l].astype(np.float32), (128, 1))
        in_maps.append(m)

    bkr = bass_utils.run_bass_kernel_spmd(
        nc, in_maps, core_ids=list(range(ndev)), trace=trace)
    out = np.concatenate([r["out"] for r in bkr.results], axis=0)
    return out, (bkr, nc, in_maps)


def _mini_nc(ndev):
    """Tiny calibration kernel: measures pure dispatch/tunnel overhead."""
    nc = bacc.Bacc("TRN2", target_bir_lowering=False, debug=False,
                   num_devices=ndev)
    src = nc.dram_tensor("src", [128, 128], F32, kind="ExternalInput")
    outd = nc.dram_tensor("out", [128, 128], F32, kind="ExternalOutput")
    from contextlib import ExitStack as _ES
    with _ES() as ctx:
        tc = ctx.enter_context(TileContext(nc))
        pool = ctx.enter_context(tc.tile_pool(name="p", bufs=1))
        t = pool.tile([128, 128], F32, tag="t", name="t")
        nc.sync.dma_start(t[:, :], src.ap())
        nc.sync.dma_start(outd.ap(), t[:, :])
    nc.compile()
    return nc, [{"src": np.zeros((128, 128), np.float32)} for _ in range(ndev)]


def timed_exec(nc, in_maps, ndev, reps=(1, 5), n_timings=3):
    """Estimate pure NEFF exec time by running M chained executions inside one
    jit call and differencing wall times (cancels dispatch/tunnel overhead)."""
    import time
    import jax
    import numpy as np_
    from jax.sharding import Mesh, PartitionSpec
    from jax.experimental.shard_map import shard_map
    from concourse import bass2jax as b2j
    import concourse.mybir as mb

    b2j.install_neuronx_cc_hook()
    partition_name = nc.partition_id_tensor.name if nc.partition_id_tensor else None
    in_names, out_names, out_avals, zero_shapes = [], [], [], []
    for alloc in nc.m.functions[0].allocations:
        if not isinstance(alloc, mb.MemoryLocationSet):
            continue
        name = alloc.memorylocations[0].name
        if alloc.kind == "ExternalInput":
            if name != partition_name:
                in_names.append(name)
        elif alloc.kind == "ExternalOutput":
            out_names.append(name)
            shape = tuple(alloc.tensor_shape)
            dtype = mb.dt.np(alloc.dtype)
            out_avals.append(jax.core.ShapedArray(shape, dtype))
            zero_shapes.append((shape, dtype))
    n_params, n_outs = len(in_names), len(out_avals)
    all_in_names = in_names + out_names + ([partition_name] if partition_name else [])

    devices = jax.devices()[:ndev]
    mesh = Mesh(np.array(devices), ("core",))
    per_core = [[np.asarray(m[name]) for name in in_names] for m in in_maps]
    concat_in = [np.concatenate([per_core[c][i] for c in range(ndev)], axis=0)
                 for i in range(n_params)]

    def _body(*args):
        operands = list(args)
        if partition_name is not None:
            operands.append(b2j.partition_id_tensor())
        return tuple(b2j._bass_exec_p.bind(
            *operands,
            out_avals=tuple(out_avals),
            in_names=tuple(all_in_names),
            out_names=tuple(out_names),
            lowering_input_output_aliases=(),
            sim_require_finite=True,
            sim_require_nnan=True,
            nc=nc,
        ))

    donate = tuple(range(n_params, n_params + n_outs))
    in_specs = (PartitionSpec("core"),) * (n_params + n_outs)
    out_specs = (PartitionSpec("core"),) * n_outs
    fn = jax.jit(shard_map(_body, mesh=mesh, in_specs=in_specs,
                           out_specs=out_specs, check_rep=False),
                 donate_argnums=donate, keep_unused=True)

    from jax.sharding import NamedSharding
    shard = NamedSharding(mesh, PartitionSpec("core"))

    def zeros_dev():
        z = [jax.device_put(np_.zeros((ndev * s[0], *s[1:]), d), shard)
             for (s, d) in zero_shapes]
        jax.block_until_ready(z)
        return z

    dev_in = [jax.device_put(a, shard) for a in concat_in]
    jax.block_until_ready(dev_in)
    r = fn(*dev_in, *zeros_dev())       # warmup + compile
    jax.block_until_ready(r)
    samples = []
    for _ in range(n_timings):
        z = zeros_dev()
        t0 = time.perf_counter()
        r = fn(*dev_in, *z)
        jax.block_until_ready(r)
        samples.append(time.perf_counter() - t0)
    return min(samples), samples


def kernel(**inputs):
    x = np.asarray(inputs["x"], np.float32)
    hei = np.asarray(inputs["hyperedge_index"]).astype(np.int64)
    hw = np.asarray(inputs["hyperedge_weight"], np.float32)
    Ws = [np.asarray(inputs[f"W{i}"], np.float32) for i in (1, 2, 3)]
    bs = [np.asarray(inputs[f"b{i}"], np.float32) for i in (1, 2, 3)]
    out, _ = run(x, hei[0], hei[1], hw, Ws, bs,
                 n_nodes=N_FULL, n_edges=M_FULL)
    return out.astype(np.float32)
